# revision 11
# baseline (speedup 1.0000x reference)
"""Trainium2 Bass kernel for nn_CRModule (retrieval_knn).

reference:
    xf = x.reshape(4096, 4096); xa = xf[:, ::2]; xb = xf[:, 1::2]   # [T=4096, 2048]
    sq[i,j] = |xa[:,i]|^2 + |xb[:,j]|^2 - 2 * xa[:,i].xb[:,j]
    wsum = fc_weight.sum(0); wa = wsum[::2]; wb = wsum[1::2]
    scores[i,j] = ((wa[i]+wb[j]) * sqrt(max(sq,0)))**2
                = (wa[i]+wb[j])**2 * max(sq[i,j], 0)     # sqrt cancels

v3 strategy (single SPMD launch, 2x4 output grid):
  Core d (r=d>>2, c=d&3) owns a [1024, 512] block of scores:
    rows  = 1024r + (256(d&3) + li) % 1024   (own na/wa shard first)
    cols  = 512c  + (256r + lj) % 512        (own nb/wb shard first)
  Main matmul (-2a)^T b in fp8 e4m3, DoubleRow, 512-wide rhs (one mm per
  (m, kpair)). fc column sums split DVE (o-innermost chunks) + GpSimd
  (o-middle chunks), partition-reduced by one f32 PE matmul. Norm shards
  (256 ch) via ScalarE squares + ones-matmuls. One AllGather ships
  [na|nb|wa|wb] (4KB); post-gather reads use partition_id-derived dynamic
  DRAM offsets split across Scalar and Sync HWDGE queues. Epilogue fused
  in PSUM: out = max(ps + na + nb, 0) * (wa + wb)^2, with the +na+nb STT
  on GpSimd and the relu*w2 STT on DVE.
  DMA order: xb, xan -> fcs -> xar, so the trailing xar stream overlaps
  the w AllGather.
"""

import numpy as np
import ml_dtypes

import concourse.bass as bass
import concourse.tile as tile
from concourse import bacc, mybir
from concourse.bass_utils import run_bass_kernel_spmd

BF16 = mybir.dt.bfloat16
F32 = mybir.dt.float32
FP16 = mybir.dt.float16
FP8 = mybir.dt.float8e4
NP_FP8 = ml_dtypes.float8_e4m3
ALU = mybir.AluOpType
DR = mybir.MatmulPerfMode.DoubleRow

D = 8
T = 4096
C = 4096
KT = 32          # 128-row k-tiles
KK = 16          # DoubleRow k-pairs
CA = 2048
MB = 1024        # output rows per core
NBC = 512        # output cols per core
O = 12288
OCV = 6          # fc chunks for DVE   (o innermost)
OCG = 6          # fc chunks for GpSimd (o middle)

_cache = {}


def _build():
    nc = bacc.Bacc("TRN2", target_bir_lowering=False, debug=False, num_devices=D)
    xan_d = nc.dram_tensor("xan", [128, KT, 256], FP8, kind="ExternalInput").ap()
    xar_d = nc.dram_tensor("xar", [128, KT, 768], FP8, kind="ExternalInput").ap()
    xb_d = nc.dram_tensor("xb", [128, KT, 512], FP8, kind="ExternalInput").ap()
    fcv_d = nc.dram_tensor("fcv", [128, OCV, 512, 8], FP16, kind="ExternalInput").ap()
    fcg_d = nc.dram_tensor("fcg", [128, OCG, 8, 512], FP16, kind="ExternalInput").ap()
    out_d = nc.dram_tensor("scores", [MB, NBC], F32, kind="ExternalOutput").ap()
    pk_in = nc.dram_tensor("pk_in", [1, 1024], F32).ap()
    pk_sh = nc.dram_tensor("pk_sh", [D, 1024], F32, addr_space="Shared").ap()
    grp = [list(range(D))]

    with tile.TileContext(nc) as tc:
        with (
            tc.tile_pool(name="xres", bufs=1) as xres,
            tc.tile_pool(name="fcp", bufs=3) as fcp,
            tc.tile_pool(name="fgp", bufs=3) as fgp,
            tc.tile_pool(name="x2p", bufs=2) as x2p,
            tc.tile_pool(name="small", bufs=1) as small,
            tc.tile_pool(name="w2p", bufs=2) as w2p,
            tc.tile_pool(name="outp", bufs=2) as outp,
            tc.tile_pool(name="psmain", bufs=1, space="PSUM") as psmain,
            tc.tile_pool(name="pse", bufs=1, space="PSUM") as pse,
        ):
            # ---- dynamic-offset registers, hoisted to the very start ----
            pid_s = nc.scalar.partition_id()
            s_r4 = pid_s & 4
            s_cb = pid_s & 3
            nw_offs = []
            for t in range(4):
                k_t = s_r4 | ((s_cb + t) & 3)
                nw_offs.append(k_t << 10)
            pid_y = nc.sync.partition_id()
            y_cb = pid_y & 3
            y_r1 = pid_y >> 2
            bc_offs = []
            for h in range(2):
                u = (y_r1 + h) & 1
                k_nb = y_cb + (u << 2)
                k_wb = (y_cb << 1) + u
                bc_offs.append(((k_nb << 10) + 512, (k_wb << 10) + 768))

            # ---- DMA emission order = arrival priority ----
            xb_t, xan_t = [], []
            for g in range(2):
                xb_c = xres.tile([128, 16, 512], FP8, name=f"xb{g}", tag=f"xb{g}")
                nc.sync.dma_start(xb_c[:], xb_d[:, 16 * g:16 * (g + 1), :])
                xb_t.append(xb_c)
            for g in range(2):
                xa_c = xres.tile([128, 16, 256], FP8, name=f"xan{g}", tag=f"xan{g}")
                nc.sync.dma_start(xa_c[:], xan_d[:, 16 * g:16 * (g + 1), :])
                xan_t.append(xa_c)
            fcv_t, fcg_t = [], []
            for oc in range(OCV + OCG):
                if oc % 2 == 0:
                    f = fcp.tile([128, 512, 8], FP16, name=f"fcv{oc}", tag="fcv")
                    nc.sync.dma_start(f[:], fcv_d[:, oc // 2, :, :])
                    fcv_t.append(f)
                else:
                    f = fgp.tile([128, 8, 512], FP16, name=f"fcg{oc}", tag="fcg")
                    nc.sync.dma_start(f[:], fcg_d[:, oc // 2, :, :])
                    fcg_t.append(f)
            xar_t = []
            for g in range(4):
                x_c = xres.tile([128, 8, 768], FP8, name=f"xar{g}", tag=f"xar{g}")
                nc.sync.dma_start(x_c[:], xar_d[:, 8 * g:8 * (g + 1), :])
                xar_t.append(x_c)

            ones = small.tile([128, 1], BF16)
            nc.vector.memset(ones[:], 1.0)
            quarter = small.tile([128, 1], BF16)
            nc.vector.memset(quarter[:], 0.25)
            onesf = small.tile([128, 1], F32)
            nc.vector.memset(onesf[:], 1.0)

            pk = small.tile([1, 1024], F32)

            # ---- norm chains: nb then na (PE + ScalarE squares) ----
            nb_ps = pse.tile([1, 256], F32, name="nb_ps", tag="pse")
            for g in range(2):
                x2b = x2p.tile([128, 16, 256], BF16, name="x2b", tag="x2b")
                nc.scalar.square(x2b[:], xb_t[g][:, :, 0:256])
                for i in range(16):
                    kt = 16 * g + i
                    nc.tensor.matmul(nb_ps[:], ones[:], x2b[:, i, :],
                                     start=(kt == 0), stop=(kt == KT - 1))
            nc.vector.tensor_copy(pk[0:1, 512:768], nb_ps[:])

            na_ps = pse.tile([1, 256], F32, name="na_ps", tag="pse")
            for g in range(2):
                x2a = x2p.tile([128, 16, 256], BF16, name="x2a", tag="x2a")
                nc.scalar.square(x2a[:], xan_t[g][:])
                for i in range(16):
                    kt = 16 * g + i
                    nc.tensor.matmul(na_ps[:], quarter[:], x2a[:, i, :],
                                     start=(kt == 0), stop=(kt == KT - 1))
            nc.vector.tensor_copy(pk[0:1, 0:256], na_ps[:])

            # ---- fc accumulation: DVE half + GpSimd half ----
            acc = small.tile([128, 512], F32)
            red = small.tile([128, 512], F32)
            for i, f in enumerate(fcv_t):
                dst = acc if i == 0 else red
                nc.vector.tensor_reduce(dst[:], f[:],
                                        axis=mybir.AxisListType.X, op=ALU.add)
                if i:
                    nc.vector.scalar_tensor_tensor(
                        acc[:], acc[:], 0.0, red[:], op0=ALU.bypass, op1=ALU.add)
            accg = small.tile([128, 512], F32)
            g1 = small.tile([128, 4, 512], F32)
            g2 = small.tile([128, 2, 512], F32)
            for i, f in enumerate(fcg_t):
                nc.gpsimd.tensor_add(g1[:], f[:, 0:4, :], f[:, 4:8, :])
                nc.gpsimd.tensor_add(g2[:], g1[:, 0:2, :], g1[:, 2:4, :])
                if i == 0:
                    nc.gpsimd.tensor_add(accg[:], g2[:, 0, :], g2[:, 1, :])
                else:
                    nc.gpsimd.tensor_add(g2[:, 0, :], g2[:, 0, :], g2[:, 1, :])
                    nc.gpsimd.tensor_add(accg[:], accg[:], g2[:, 0, :])
            nc.vector.scalar_tensor_tensor(
                acc[:], acc[:], 0.0, accg[:], op0=ALU.bypass, op1=ALU.add)

            # ---- main mm (512-wide DoubleRow) ----
            ps7 = psmain.tile([128, 7, 512], F32, name="ps7", tag="ps7")

            def lhs(m, kk):
                if m < 2:
                    g, s = divmod(kk, 8)
                    return xan_t[g][:, 2 * s:2 * s + 2, 128 * m:128 * (m + 1)]
                g, s = divmod(kk, 4)
                return xar_t[g][:, 2 * s:2 * s + 2, 128 * (m - 2):128 * (m - 1)]

            def rhs(kk):
                g, s = divmod(kk, 8)
                return xb_t[g][:, 2 * s:2 * s + 2, :]

            for kk in range(KK):
                for m in (0, 1):
                    nc.tensor.matmul(ps7[:, m, :], lhs(m, kk), rhs(kk),
                                     start=(kk == 0), stop=(kk == KK - 1),
                                     perf_mode=DR)

            # ---- w partition-reduce + collective ----
            w_ps = pse.tile([1, 512], F32, name="w_ps", tag="pse")
            nc.tensor.matmul(w_ps[:], onesf[:], acc[:], start=True, stop=True)
            nc.vector.tensor_copy(pk[0:1, 256:512], w_ps[0:1, 0:256])
            nc.vector.tensor_copy(pk[0:1, 768:1024], w_ps[0:1, 256:512])
            nc.gpsimd.dma_start(pk_in[:], pk[:])
            nc.gpsimd.collective_compute(
                "AllGather", ALU.bypass, replica_groups=grp,
                ins=[pk_in[:]], outs=[pk_sh[:]])

            # ---- rest of main mm ----
            for kk in range(KK):
                for m in range(2, 7):
                    nc.tensor.matmul(ps7[:, m, :], lhs(m, kk), rhs(kk),
                                     start=(kk == 0), stop=(kk == KK - 1),
                                     perf_mode=DR)
            ps7b = pse.tile([128, 512], F32, name="ps7b", tag="pse")
            for kk in range(KK):
                nc.tensor.matmul(ps7b[:], lhs(7, kk), rhs(kk),
                                 start=(kk == 0), stop=(kk == KK - 1),
                                 perf_mode=DR)

            # ---- post-gather reads (dynamic DRAM offsets, 2 HWDGE queues) ----
            nw_t = []
            for t in range(4):
                nw = small.tile([128, 4], F32, name=f"nw{t}", tag=f"nw{t}")
                nc.scalar.dma_start(
                    nw[:],
                    bass.AP(tensor=pk_sh.tensor, offset=nw_offs[t],
                            ap=[[1, 128], [128, 4]]))
                nw_t.append(nw)
            nbbc = small.tile([128, 512], F32)
            wbbc = small.tile([128, 512], F32)
            for h in range(2):
                onb, owb = bc_offs[h]
                nc.sync.dma_start(
                    nbbc[:, 256 * h:256 * (h + 1)],
                    bass.AP(tensor=pk_sh.tensor, offset=onb,
                            ap=[[0, 128], [1, 256]]))
                nc.sync.dma_start(
                    wbbc[:, 256 * h:256 * (h + 1)],
                    bass.AP(tensor=pk_sh.tensor, offset=owb,
                            ap=[[0, 128], [1, 256]]))

            # ---- fused epilogue per m-tile ----
            for m in range(8):
                psm = ps7[:, m, :] if m < 7 else ps7b[:]
                nav = nw_t[m // 2][:, m % 2:m % 2 + 1]      # [128,1]
                wav = nw_t[m // 2][:, 2 + m % 2:3 + m % 2]
                w2m = w2p.tile([128, 512], F32, name="w2m", tag="w2")
                nc.scalar.activation(w2m[:], wbbc[:],
                                     mybir.ActivationFunctionType.Square,
                                     bias=wav, scale=1.0)
                nc.vector.scalar_tensor_tensor(
                    psm, psm, nav, nbbc[:], op0=ALU.add, op1=ALU.add)
                ot = outp.tile([128, 512], F32, name="ot", tag="ot")
                nc.vector.scalar_tensor_tensor(
                    ot[:], psm, 0.0, w2m[:], op0=ALU.max, op1=ALU.mult)
                nc.sync.dma_start(out_d[128 * m:128 * (m + 1), :], ot[:])

    nc.compile()
    return nc


def _p_major(a, np_dtype):
    """[T, cols] -> [128, T//128, cols]."""
    n = a.shape[0] // 128
    return np.ascontiguousarray(
        a.reshape(n, 128, a.shape[1]).transpose(1, 0, 2).astype(np_dtype))


def _core_geom(d):
    r, cb = d >> 2, d & 3
    rows = 1024 * r + (256 * cb + np.arange(MB)) % 1024
    cols = 512 * cb + (256 * r + np.arange(NBC)) % 512
    return rows, cols


def kernel(x, fc_weight, _trace=False):
    """Full inputs in, full [2048, 2048] scores out."""
    x = np.asarray(x, dtype=np.float32)
    fc = np.asarray(fc_weight, dtype=np.float32)
    xf = x.reshape(T, C)
    xa2 = np.ascontiguousarray(xf[:, 0::2]) * -2.0   # [T, 2048]
    xb = np.ascontiguousarray(xf[:, 1::2])

    if "v3" not in _cache:
        _cache["v3"] = _build()
    ncv = _cache["v3"]

    in_maps = []
    geoms = []
    for d in range(D):
        rows, cols = _core_geom(d)
        geoms.append((rows, cols))
        xa_blk = xa2[:, rows]
        xb_blk = xb[:, cols]
        fcd = fc[:, 512 * d:512 * (d + 1)]
        fcs = np.concatenate([fcd[:, 0::2], fcd[:, 1::2]], axis=1)  # [O, 512]
        fcs = fcs.reshape(12, 8, 128, 512).astype(np.float16)
        # even macro-chunks -> DVE (o innermost), odd -> GpSimd (o middle)
        fcv = np.ascontiguousarray(fcs[0::2].transpose(2, 0, 3, 1))  # [128,6,512,8]
        fcg = np.ascontiguousarray(fcs[1::2].transpose(2, 0, 1, 3))  # [128,6,8,512]
        in_maps.append({
            "xan": _p_major(xa_blk[:, :256], NP_FP8),
            "xar": _p_major(xa_blk[:, 256:], NP_FP8),
            "xb": _p_major(xb_blk, NP_FP8),
            "fcv": fcv,
            "fcg": fcg,
        })

    res = run_bass_kernel_spmd(ncv, in_maps, core_ids=list(range(D)), trace=_trace)
    out = np.empty((CA, CA), dtype=np.float32)
    for d in range(D):
        rows, cols = geoms[d]
        out[np.ix_(rows, cols)] = res.results[d]["scores"]
    if _trace:
        kernel.last_times = (res.exec_time_ns,)
    return out


# revision 12
# speedup vs baseline: 1.2370x; 1.2370x over previous
"""Trainium2 Bass kernel for nn_CRModule (retrieval_knn).

reference:
    xf = x.reshape(4096, 4096); xa = xf[:, ::2]; xb = xf[:, 1::2]   # [T=4096, 2048]
    sq[i,j] = |xa[:,i]|^2 + |xb[:,j]|^2 - 2 * xa[:,i].xb[:,j]
    wsum = fc_weight.sum(0); wa = wsum[::2]; wb = wsum[1::2]
    scores[i,j] = ((wa[i]+wb[j]) * sqrt(max(sq,0)))**2
                = (wa[i]+wb[j])**2 * max(sq[i,j], 0)     # sqrt cancels

v4 strategy (single SPMD launch, 2x4 output grid):
  Core d (r=d>>2, c=d&3) owns a [1024, 512] block of scores:
    rows  = 1024r + (256(d&3) + li) % 1024   (own na/wa shard first)
    cols  = 512c  + (256r + lj) % 512        (own nb/wb shard first)
  Main matmul (-2a)^T b in fp8 e4m3, DoubleRow, 512-wide rhs. fc column
  sums on DVE: fp16 chunk tensor_reduce (4x perf mode) + f32 combine,
  partition-reduced by one f32 PE matmul. Norm shards (256 ch) via
  ScalarE squares + ones-matmuls. A dummy prewarm AllGather absorbs CC
  cold-start + core skew; the real AllGather ships [na|wa|nb|wb] (4KB).
  Post-gather: na/nb enter the psum chains as K=1 fp16 matmuls
  (lhsT/rhs line vectors), wa transposes to per-partition via K=1
  matmuls, so the epilogue is just ACT w2 + one DVE STT per m-tile:
  out = max(ps, 0) * (wa + wb)^2.
  DMA order: xb, xan -> fcs -> xar, so the trailing xar stream plus the
  prewarm keep the w AllGather off the critical path.
"""

import numpy as np
import ml_dtypes

import concourse.bass as bass
import concourse.tile as tile
from concourse import bacc, mybir
from concourse.bass_utils import run_bass_kernel_spmd

BF16 = mybir.dt.bfloat16
F32 = mybir.dt.float32
FP16 = mybir.dt.float16
FP8 = mybir.dt.float8e4
NP_FP8 = ml_dtypes.float8_e4m3
ALU = mybir.AluOpType
DR = mybir.MatmulPerfMode.DoubleRow

D = 8
T = 4096
C = 4096
KT = 32          # 128-row k-tiles
KK = 16          # DoubleRow k-pairs
CA = 2048
MB = 1024        # output rows per core
NBC = 512        # output cols per core
O = 12288
OC = 12          # fc chunks (1024 rows each)

_cache = {}


def _build():
    nc = bacc.Bacc("TRN2", target_bir_lowering=False, debug=False, num_devices=D)
    xan_d = nc.dram_tensor("xan", [128, KT, 256], FP8, kind="ExternalInput").ap()
    xar_d = nc.dram_tensor("xar", [128, KT, 768], FP8, kind="ExternalInput").ap()
    xb_d = nc.dram_tensor("xb", [128, KT, 512], FP8, kind="ExternalInput").ap()
    fcv_d = nc.dram_tensor("fcv", [128, OC, 512, 8], FP16, kind="ExternalInput").ap()
    out_d = nc.dram_tensor("scores", [MB, NBC], F32, kind="ExternalOutput").ap()
    pk_in = nc.dram_tensor("pk_in", [1, 1024], F32).ap()
    pk_sh = nc.dram_tensor("pk_sh", [D, 1024], F32, addr_space="Shared").ap()
    warm_in = nc.dram_tensor("warm_in", [1, 8], F32).ap()
    warm_sh = nc.dram_tensor("warm_sh", [D, 8], F32, addr_space="Shared").ap()
    grp = [list(range(D))]

    with tile.TileContext(nc) as tc:
        with (
            tc.tile_pool(name="xres", bufs=1) as xres,
            tc.tile_pool(name="fcp", bufs=3) as fcp,
            tc.tile_pool(name="x2p", bufs=2) as x2p,
            tc.tile_pool(name="small", bufs=1) as small,
            tc.tile_pool(name="w2p", bufs=2) as w2p,
            tc.tile_pool(name="outp", bufs=2) as outp,
            tc.tile_pool(name="psmain", bufs=1, space="PSUM") as psmain,
            tc.tile_pool(name="pse", bufs=1, space="PSUM") as pse,
        ):
            # ---- dynamic-offset registers, hoisted to the very start ----
            pid_s = nc.scalar.partition_id()
            s_r4 = pid_s & 4
            s_cb = pid_s & 3
            nw_offs = []
            for t in range(4):
                k_t = s_r4 | ((s_cb + t) & 3)
                nw_offs.append(k_t << 10)
            pid_y = nc.sync.partition_id()
            y_cb = pid_y & 3
            y_r1 = pid_y >> 2
            bc_offs = []
            for h in range(2):
                u = (y_r1 + h) & 1
                k_nb = y_cb + (u << 2)
                k_wb = (y_cb << 1) + u
                bc_offs.append(((k_nb << 10) + 512, (k_wb << 10) + 768))

            # ---- prewarm collective: absorbs CC cold-start + core skew ----
            warm = small.tile([1, 8], F32)
            nc.gpsimd.memset(warm[:], 1.0)
            nc.gpsimd.dma_start(warm_in[:], warm[:])
            nc.gpsimd.collective_compute(
                "AllGather", ALU.bypass, replica_groups=grp,
                ins=[warm_in[:]], outs=[warm_sh[:]])

            # ---- DMA emission order = arrival priority ----
            xb_t, xan_t = [], []
            for g in range(2):
                xb_c = xres.tile([128, 16, 512], FP8, name=f"xb{g}", tag=f"xb{g}")
                nc.sync.dma_start(xb_c[:], xb_d[:, 16 * g:16 * (g + 1), :])
                xb_t.append(xb_c)
            for g in range(2):
                xa_c = xres.tile([128, 16, 256], FP8, name=f"xan{g}", tag=f"xan{g}")
                nc.sync.dma_start(xa_c[:], xan_d[:, 16 * g:16 * (g + 1), :])
                xan_t.append(xa_c)
            fct = []
            for oc in range(OC):
                f = fcp.tile([128, 512, 8], FP16, name=f"fc{oc}", tag="fc")
                nc.sync.dma_start(f[:], fcv_d[:, oc, :, :])
                fct.append(f)
            xar_t = []
            for g in range(4):
                x_c = xres.tile([128, 8, 768], FP8, name=f"xar{g}", tag=f"xar{g}")
                nc.sync.dma_start(x_c[:], xar_d[:, 8 * g:8 * (g + 1), :])
                xar_t.append(x_c)

            ones = small.tile([128, 1], BF16)
            nc.vector.memset(ones[:], 1.0)
            quarter = small.tile([128, 1], BF16)
            nc.vector.memset(quarter[:], 0.25)
            onesf = small.tile([128, 1], F32)
            nc.vector.memset(onesf[:], 1.0)
            onesh1 = small.tile([1, 128], FP16)
            nc.vector.memset(onesh1[:], 1.0)
            onesn = small.tile([1, NBC], FP16)
            nc.vector.memset(onesn[:], 1.0)
            onef1 = small.tile([1, 1], F32)
            nc.vector.memset(onef1[:], 1.0)

            pk = small.tile([1, 1024], F32)

            # ---- norm chains: nb then na (PE + ScalarE squares) ----
            nb_ps = pse.tile([1, 256], F32, name="nb_ps", tag="pse")
            for g in range(2):
                x2b = x2p.tile([128, 16, 256], BF16, name="x2b", tag="x2b")
                nc.scalar.square(x2b[:], xb_t[g][:, :, 0:256])
                for i in range(16):
                    kt = 16 * g + i
                    nc.tensor.matmul(nb_ps[:], ones[:], x2b[:, i, :],
                                     start=(kt == 0), stop=(kt == KT - 1))
            nc.vector.tensor_copy(pk[0:1, 512:768], nb_ps[:])

            na_ps = pse.tile([1, 256], F32, name="na_ps", tag="pse")
            for g in range(2):
                x2a = x2p.tile([128, 16, 256], BF16, name="x2a", tag="x2a")
                nc.scalar.square(x2a[:], xan_t[g][:])
                for i in range(16):
                    kt = 16 * g + i
                    nc.tensor.matmul(na_ps[:], quarter[:], x2a[:, i, :],
                                     start=(kt == 0), stop=(kt == KT - 1))
            nc.vector.tensor_copy(pk[0:1, 0:256], na_ps[:])

            # ---- fc accumulation on DVE: fp16 4x reduces + f32 combine ----
            csum = small.tile([128, OC, 512], FP16)
            acc = small.tile([128, 512], F32)
            with nc.allow_low_precision("fp16 fc chunk sums, f32 combine"):
                for i, f in enumerate(fct):
                    nc.vector.tensor_reduce(csum[:, i, :], f[:],
                                            axis=mybir.AxisListType.X, op=ALU.add)
                    if i == 1:
                        nc.vector.scalar_tensor_tensor(
                            acc[:], csum[:, 0, :], 0.0, csum[:, 1, :],
                            op0=ALU.bypass, op1=ALU.add)
                    elif i > 1:
                        nc.vector.scalar_tensor_tensor(
                            acc[:], acc[:], 0.0, csum[:, i, :],
                            op0=ALU.bypass, op1=ALU.add)

            # ---- main mm (512-wide DoubleRow); chains stay open for the
            # ---- K=1 na/nb injection matmuls after the gather ----
            ps7 = psmain.tile([128, 7, 512], F32, name="ps7", tag="ps7")

            def lhs(m, kk):
                if m < 2:
                    g, s = divmod(kk, 8)
                    return xan_t[g][:, 2 * s:2 * s + 2, 128 * m:128 * (m + 1)]
                g, s = divmod(kk, 4)
                return xar_t[g][:, 2 * s:2 * s + 2, 128 * (m - 2):128 * (m - 1)]

            def rhs(kk):
                g, s = divmod(kk, 8)
                return xb_t[g][:, 2 * s:2 * s + 2, :]

            for kk in range(KK):
                for m in (0, 1):
                    nc.tensor.matmul(ps7[:, m, :], lhs(m, kk), rhs(kk),
                                     start=(kk == 0), stop=False, perf_mode=DR)

            # ---- w partition-reduce + collective ----
            w_ps = pse.tile([1, 512], F32, name="w_ps", tag="pse")
            nc.tensor.matmul(w_ps[:], onesf[:], acc[:], start=True, stop=True)
            nc.vector.tensor_copy(pk[0:1, 256:512], w_ps[0:1, 0:256])
            nc.vector.tensor_copy(pk[0:1, 768:1024], w_ps[0:1, 256:512])
            nc.gpsimd.dma_start(pk_in[:], pk[:])
            nc.gpsimd.collective_compute(
                "AllGather", ALU.bypass, replica_groups=grp,
                ins=[pk_in[:]], outs=[pk_sh[:]])

            # ---- rest of main mm ----
            for kk in range(KK):
                for m in range(2, 7):
                    nc.tensor.matmul(ps7[:, m, :], lhs(m, kk), rhs(kk),
                                     start=(kk == 0), stop=False, perf_mode=DR)

            # ---- post-gather reads (dynamic DRAM offsets, 2 HWDGE queues) --
            # lal[0, t, :] = [na shard | wa shard] of r-block member (cb+t)&3
            lal = small.tile([1, 4, 512], F32)
            for t in range(4):
                nc.scalar.dma_start(
                    lal[:, t, :],
                    bass.AP(tensor=pk_sh.tensor, offset=nw_offs[t],
                            ap=[[1, 1], [1, 512]]))
            nblf = small.tile([1, 512], F32)
            wbbc = small.tile([128, 512], F32)
            for h in range(2):
                onb, owb = bc_offs[h]
                nc.sync.dma_start(
                    nblf[:, 256 * h:256 * (h + 1)],
                    bass.AP(tensor=pk_sh.tensor, offset=onb,
                            ap=[[1, 1], [1, 256]]))
                nc.sync.dma_start(
                    wbbc[:, 256 * h:256 * (h + 1)],
                    bass.AP(tensor=pk_sh.tensor, offset=owb,
                            ap=[[0, 128], [1, 256]]))
            # fp16 line vectors for the K=1 injection matmuls
            nal_h = small.tile([1, 4, 512], FP16)
            nc.scalar.copy(nal_h[:], lal[:])       # wa halves unused but cheap
            nbl_h = small.tile([1, 512], FP16)
            nc.scalar.copy(nbl_h[:], nblf[:])

            # ---- wa -> per-partition via K=1 transpose matmuls ----
            wat_ps = pse.tile([128, 8], F32, name="wat_ps", tag="pse")
            for m in range(8):
                nc.tensor.matmul(
                    wat_ps[:, m:m + 1],
                    lal[0:1, m // 2, 256 + 128 * (m % 2):256 + 128 * (m % 2) + 128],
                    onef1[:], start=(m == 0), stop=(m == 7),
                    skip_group_check=True)
            wat = small.tile([128, 8], F32)
            nc.vector.tensor_copy(wat[:], wat_ps[:])

            # ---- m7 chain (pse bank, after wat_ps drained) ----
            ps7b = pse.tile([128, 512], F32, name="ps7b", tag="pse")
            for kk in range(KK):
                nc.tensor.matmul(ps7b[:], lhs(7, kk), rhs(kk),
                                 start=(kk == 0), stop=False, perf_mode=DR)

            # ---- inject na/nb into each psum chain, then epilogue ----
            for m in range(8):
                psm = ps7[:, m, :] if m < 7 else ps7b[:]
                nc.tensor.matmul(psm, onesh1[:], nbl_h[:],
                                 start=False, stop=False, skip_group_check=True)
                nc.tensor.matmul(
                    psm, nal_h[0:1, m // 2, 128 * (m % 2):128 * (m % 2) + 128],
                    onesn[:], start=False, stop=True, skip_group_check=True)
                w2m = w2p.tile([128, 512], F32, name="w2m", tag="w2")
                nc.scalar.activation(w2m[:], wbbc[:],
                                     mybir.ActivationFunctionType.Square,
                                     bias=wat[:, m:m + 1], scale=1.0)
                ot = outp.tile([128, 512], F32, name="ot", tag="ot")
                nc.vector.scalar_tensor_tensor(
                    ot[:], psm, 0.0, w2m[:], op0=ALU.max, op1=ALU.mult)
                nc.sync.dma_start(out_d[128 * m:128 * (m + 1), :], ot[:])

    nc.compile()
    return nc


def _p_major(a, np_dtype):
    """[T, cols] -> [128, T//128, cols]."""
    n = a.shape[0] // 128
    return np.ascontiguousarray(
        a.reshape(n, 128, a.shape[1]).transpose(1, 0, 2).astype(np_dtype))


def _core_geom(d):
    r, cb = d >> 2, d & 3
    rows = 1024 * r + (256 * cb + np.arange(MB)) % 1024
    cols = 512 * cb + (256 * r + np.arange(NBC)) % 512
    return rows, cols


def kernel(x, fc_weight, _trace=False):
    """Full inputs in, full [2048, 2048] scores out."""
    x = np.asarray(x, dtype=np.float32)
    fc = np.asarray(fc_weight, dtype=np.float32)
    xf = x.reshape(T, C)
    xa2 = np.ascontiguousarray(xf[:, 0::2]) * -2.0   # [T, 2048]
    xb = np.ascontiguousarray(xf[:, 1::2])

    if "v4" not in _cache:
        _cache["v4"] = _build()
    ncv = _cache["v4"]

    in_maps = []
    geoms = []
    for d in range(D):
        rows, cols = _core_geom(d)
        geoms.append((rows, cols))
        xa_blk = xa2[:, rows]
        xb_blk = xb[:, cols]
        fcd = fc[:, 512 * d:512 * (d + 1)]
        fcs = np.concatenate([fcd[:, 0::2], fcd[:, 1::2]], axis=1)  # [O, 512]
        fcv = np.ascontiguousarray(
            fcs.reshape(OC, 8, 128, 512).transpose(2, 0, 3, 1)
        ).astype(np.float16)                                        # [128,12,512,8]
        in_maps.append({
            "xan": _p_major(xa_blk[:, :256], NP_FP8),
            "xar": _p_major(xa_blk[:, 256:], NP_FP8),
            "xb": _p_major(xb_blk, NP_FP8),
            "fcv": fcv,
        })

    res = run_bass_kernel_spmd(ncv, in_maps, core_ids=list(range(D)), trace=_trace)
    out = np.empty((CA, CA), dtype=np.float32)
    for d in range(D):
        rows, cols = geoms[d]
        out[np.ix_(rows, cols)] = res.results[d]["scores"]
    if _trace:
        kernel.last_times = (res.exec_time_ns,)
    return out


# revision 15
# speedup vs baseline: 1.3489x; 1.0904x over previous
"""Trainium2 Bass kernel for nn_CRModule (retrieval_knn).

reference:
    xf = x.reshape(4096, 4096); xa = xf[:, ::2]; xb = xf[:, 1::2]   # [T=4096, 2048]
    sq[i,j] = |xa[:,i]|^2 + |xb[:,j]|^2 - 2 * xa[:,i].xb[:,j]
    wsum = fc_weight.sum(0); wa = wsum[::2]; wb = wsum[1::2]
    scores[i,j] = ((wa[i]+wb[j]) * sqrt(max(sq,0)))**2
                = (wa[i]+wb[j])**2 * max(sq[i,j], 0)     # sqrt cancels

v4 strategy (single SPMD launch, 2x4 output grid):
  Core d (r=d>>2, c=d&3) owns a [1024, 512] block of scores:
    rows  = 1024r + (256(d&3) + li) % 1024   (own na/wa shard first)
    cols  = 512c  + (256r + lj) % 512        (own nb/wb shard first)
  Main matmul (-2a)^T b in fp8 e4m3, DoubleRow, 512-wide rhs. fc column
  sums on DVE: fp16 chunk tensor_reduce (4x perf mode) + f32 combine,
  partition-reduced by one f32 PE matmul. Norm shards (256 ch) via
  ScalarE squares + ones-matmuls. A dummy prewarm AllGather absorbs CC
  cold-start + core skew; the real AllGather ships [na|wa|nb|wb] (4KB).
  Post-gather: na/nb enter the psum chains as K=1 fp16 matmuls
  (lhsT/rhs line vectors), wa transposes to per-partition via K=1
  matmuls, so the epilogue is just ACT w2 + one DVE STT per m-tile:
  out = max(ps, 0) * (wa + wb)^2.
  DMA order: xb, xan -> fcs -> xar, so the trailing xar stream plus the
  prewarm keep the w AllGather off the critical path.
"""

import numpy as np
import ml_dtypes

import concourse.bass as bass
import concourse.tile as tile
from concourse import bacc, mybir
from concourse.bass_utils import run_bass_kernel_spmd

BF16 = mybir.dt.bfloat16
F32 = mybir.dt.float32
FP16 = mybir.dt.float16
FP8 = mybir.dt.float8e4
NP_FP8 = ml_dtypes.float8_e4m3
ALU = mybir.AluOpType
DR = mybir.MatmulPerfMode.DoubleRow

D = 8
T = 4096
C = 4096
KT = 32          # 128-row k-tiles
KK = 16          # DoubleRow k-pairs
CA = 2048
MB = 1024        # output rows per core
NBC = 512        # output cols per core
O = 12288
OCV = 8          # fc chunks on DVE (o innermost)
OCG = 4          # fc chunks on GpSimd (o middle)

_cache = {}


def _build():
    nc = bacc.Bacc("TRN2", target_bir_lowering=False, debug=False, num_devices=D)
    xan_d = nc.dram_tensor("xan", [128, KT, 256], FP8, kind="ExternalInput").ap()
    xar_d = nc.dram_tensor("xar", [128, KT, 768], FP8, kind="ExternalInput").ap()
    xb_d = nc.dram_tensor("xb", [128, KT, 512], FP8, kind="ExternalInput").ap()
    fcv_d = nc.dram_tensor("fcv", [128, OCV, 512, 8], FP16, kind="ExternalInput").ap()
    fcg_d = nc.dram_tensor("fcg", [128, OCG, 8, 512], FP16, kind="ExternalInput").ap()
    wasc = nc.dram_tensor("wasc", [1, 1024], FP16).ap()
    out_d = nc.dram_tensor("scores", [MB, NBC], F32, kind="ExternalOutput").ap()
    pk_in = nc.dram_tensor("pk_in", [1, 1024], F32).ap()
    pk_sh = nc.dram_tensor("pk_sh", [D, 1024], F32, addr_space="Shared").ap()
    warm_in = nc.dram_tensor("warm_in", [1, 8], F32).ap()
    warm_sh = nc.dram_tensor("warm_sh", [D, 8], F32, addr_space="Shared").ap()
    grp = [list(range(D))]

    with tile.TileContext(nc) as tc:
        with (
            tc.tile_pool(name="xres", bufs=1) as xres,
            tc.tile_pool(name="fcp", bufs=3) as fcp,
            tc.tile_pool(name="fgp", bufs=2) as fgp,
            tc.tile_pool(name="x2p", bufs=2) as x2p,
            tc.tile_pool(name="small", bufs=1) as small,
            tc.tile_pool(name="w2p", bufs=2) as w2p,
            tc.tile_pool(name="outp", bufs=2) as outp,
            tc.tile_pool(name="psmain", bufs=1, space="PSUM") as psmain,
            tc.tile_pool(name="pse", bufs=1, space="PSUM") as pse,
        ):
            # ---- dynamic-offset registers, hoisted to the very start ----
            pid_s = nc.scalar.partition_id()
            s_r4 = pid_s & 4
            s_cb = pid_s & 3
            nw_offs = []
            for t in range(4):
                k_t = s_r4 | ((s_cb + t) & 3)
                nw_offs.append(k_t << 10)
            pid_y = nc.sync.partition_id()
            y_cb = pid_y & 3
            y_r1 = pid_y >> 2
            bc_offs = []
            for h in range(2):
                u = (y_r1 + h) & 1
                k_nb = y_cb + (u << 2)
                k_wb = (y_cb << 1) + u
                bc_offs.append(((k_nb << 10) + 512, (k_wb << 10) + 768))

            # ---- prewarm collective: absorbs CC cold-start + core skew ----
            warm = small.tile([1, 8], F32)
            nc.gpsimd.memset(warm[:], 1.0)
            nc.gpsimd.dma_start(warm_in[:], warm[:])
            nc.gpsimd.collective_compute(
                "AllGather", ALU.bypass, replica_groups=grp,
                ins=[warm_in[:]], outs=[warm_sh[:]])

            # ---- DMA emission order = arrival priority ----
            xb_t, xan_t = [], []
            for g in range(2):
                xb_c = xres.tile([128, 16, 512], FP8, name=f"xb{g}", tag=f"xb{g}")
                nc.sync.dma_start(xb_c[:], xb_d[:, 16 * g:16 * (g + 1), :])
                xb_t.append(xb_c)
            for g in range(2):
                xa_c = xres.tile([128, 16, 256], FP8, name=f"xan{g}", tag=f"xan{g}")
                nc.sync.dma_start(xa_c[:], xan_d[:, 16 * g:16 * (g + 1), :])
                xan_t.append(xa_c)
            fct, fgt = [], []
            for oc in range(12):
                if oc % 3 == 2:
                    f = fgp.tile([128, 8, 512], FP16, name=f"fcg{oc}", tag="fcg")
                    nc.sync.dma_start(f[:], fcg_d[:, oc // 3, :, :])
                    fgt.append(f)
                else:
                    f = fcp.tile([128, 512, 8], FP16, name=f"fcv{oc}", tag="fcv")
                    nc.sync.dma_start(f[:], fcv_d[:, oc - oc // 3 - (oc % 3 == 2), :, :])
                    fct.append(f)
            xar_t = []
            for g in range(4):
                x_c = xres.tile([128, 8, 768], FP8, name=f"xar{g}", tag=f"xar{g}")
                nc.sync.dma_start(x_c[:], xar_d[:, 8 * g:8 * (g + 1), :])
                xar_t.append(x_c)

            ones = small.tile([128, 1], BF16)
            nc.vector.memset(ones[:], 1.0)
            quarter = small.tile([128, 1], BF16)
            nc.vector.memset(quarter[:], 0.25)
            onesf = small.tile([128, 1], F32)
            nc.vector.memset(onesf[:], 1.0)

            pk = small.tile([1, 1024], F32)

            # ---- norm chains: nb then na (PE + ScalarE squares) ----
            nb_ps = pse.tile([1, 256], F32, name="nb_ps", tag="pse")
            for g in range(2):
                x2b = x2p.tile([128, 16, 256], BF16, name="x2b", tag="x2b")
                nc.scalar.square(x2b[:], xb_t[g][:, :, 0:256])
                for i in range(16):
                    kt = 16 * g + i
                    nc.tensor.matmul(nb_ps[:], ones[:], x2b[:, i, :],
                                     start=(kt == 0), stop=(kt == KT - 1))
            nc.vector.tensor_copy(pk[0:1, 512:768], nb_ps[:])

            na_ps = pse.tile([1, 256], F32, name="na_ps", tag="pse")
            for g in range(2):
                x2a = x2p.tile([128, 16, 256], BF16, name="x2a", tag="x2a")
                nc.scalar.square(x2a[:], xan_t[g][:])
                for i in range(16):
                    kt = 16 * g + i
                    nc.tensor.matmul(na_ps[:], quarter[:], x2a[:, i, :],
                                     start=(kt == 0), stop=(kt == KT - 1))
            nc.vector.tensor_copy(pk[0:1, 0:256], na_ps[:])

            # ---- fc accumulation: DVE 8 chunks + GpSimd 4 chunks ----
            csum = small.tile([128, OCV, 512], FP16)
            acc = small.tile([128, 512], F32)
            with nc.allow_low_precision("fp16 fc chunk sums, f32 combine"):
                for i, f in enumerate(fct):
                    nc.vector.tensor_reduce(csum[:, i, :], f[:],
                                            axis=mybir.AxisListType.X, op=ALU.add)
                    if i == 1:
                        nc.vector.scalar_tensor_tensor(
                            acc[:], csum[:, 0, :], 0.0, csum[:, 1, :],
                            op0=ALU.bypass, op1=ALU.add)
                    elif i > 1:
                        nc.vector.scalar_tensor_tensor(
                            acc[:], acc[:], 0.0, csum[:, i, :],
                            op0=ALU.bypass, op1=ALU.add)
            accg = small.tile([128, 512], F32)
            g1 = small.tile([128, 4, 512], F32)
            g2 = small.tile([128, 2, 512], F32)
            for i, f in enumerate(fgt):
                nc.gpsimd.tensor_add(g1[:], f[:, 0:4, :], f[:, 4:8, :])
                nc.gpsimd.tensor_add(g2[:], g1[:, 0:2, :], g1[:, 2:4, :])
                if i == 0:
                    nc.gpsimd.tensor_add(accg[:], g2[:, 0, :], g2[:, 1, :])
                else:
                    nc.gpsimd.tensor_add(g2[:, 0, :], g2[:, 0, :], g2[:, 1, :])
                    nc.gpsimd.tensor_add(accg[:], accg[:], g2[:, 0, :])
            nc.vector.scalar_tensor_tensor(
                acc[:], acc[:], 0.0, accg[:], op0=ALU.bypass, op1=ALU.add)

            # ---- main mm (512-wide DoubleRow); chains stay open for the
            # ---- K=1 na/nb injection matmuls after the gather ----
            ps7 = psmain.tile([128, 7, 512], F32, name="ps7", tag="ps7")

            def lhs(m, kk):
                if m < 2:
                    g, s = divmod(kk, 8)
                    return xan_t[g][:, 2 * s:2 * s + 2, 128 * m:128 * (m + 1)]
                g, s = divmod(kk, 4)
                return xar_t[g][:, 2 * s:2 * s + 2, 128 * (m - 2):128 * (m - 1)]

            def rhs(kk):
                g, s = divmod(kk, 8)
                return xb_t[g][:, 2 * s:2 * s + 2, :]

            for kk in range(KK):
                for m in (0, 1):
                    nc.tensor.matmul(ps7[:, m, :], lhs(m, kk), rhs(kk),
                                     start=(kk == 0), stop=False, perf_mode=DR)

            # ---- w partition-reduce + collective ----
            w_ps = pse.tile([1, 512], F32, name="w_ps", tag="pse")
            nc.tensor.matmul(w_ps[:], onesf[:], acc[:], start=True, stop=True)
            nc.vector.tensor_copy(pk[0:1, 256:512], w_ps[0:1, 0:256])
            nc.vector.tensor_copy(pk[0:1, 768:1024], w_ps[0:1, 256:512])
            nc.gpsimd.dma_start(pk_in[:], pk[:])
            nc.gpsimd.collective_compute(
                "AllGather", ALU.bypass, replica_groups=grp,
                ins=[pk_in[:]], outs=[pk_sh[:]])

            # ---- rest of main mm ----
            for kk in range(KK):
                for m in range(2, 7):
                    nc.tensor.matmul(ps7[:, m, :], lhs(m, kk), rhs(kk),
                                     start=(kk == 0), stop=False, perf_mode=DR)

            # ---- post-gather reads (dynamic DRAM offsets, 2 HWDGE queues) --
            # lal[0, t, :] = [na shard | wa shard] of r-block member (cb+t)&3
            lal = small.tile([1, 4, 512], F32)
            for t in range(4):
                nc.scalar.dma_start(
                    lal[:, t, :],
                    bass.AP(tensor=pk_sh.tensor, offset=nw_offs[t],
                            ap=[[1, 1], [1, 512]]))
            nblf = small.tile([1, 512], F32)
            wbbc = small.tile([128, 512], F32)
            for h in range(2):
                onb, owb = bc_offs[h]
                nc.sync.dma_start(
                    nblf[:, 256 * h:256 * (h + 1)],
                    bass.AP(tensor=pk_sh.tensor, offset=onb,
                            ap=[[1, 1], [1, 256]]))
                nc.sync.dma_start(
                    wbbc[:, 256 * h:256 * (h + 1)],
                    bass.AP(tensor=pk_sh.tensor, offset=owb,
                            ap=[[0, 128], [1, 256]]))
            # fp16 lines: lah = wa rows, nk2/nb2 = K=2 inject operands
            lah = small.tile([1, 4, 256], FP16)
            nc.scalar.copy(lah[:], lal[:, :, 256:512])
            nk2 = small.tile([2, 4, 256], FP16)       # row0 = na, row1 = 1
            nc.vector.memset(nk2[:], 1.0)
            nc.scalar.copy(nk2[0:1, :, :], lal[:, :, 0:256])
            nb2 = small.tile([2, 512], FP16)          # row0 = 1, row1 = nb
            nc.vector.memset(nb2[:], 1.0)
            nbl_h = small.tile([1, 512], FP16)
            nc.scalar.copy(nbl_h[:], nblf[:])
            nc.scalar.dma_start(nb2[1:2, :], nbl_h[:])

            # ---- wa -> per-partition via DRAM roundtrip (static APs) ----
            nc.scalar.dma_start(wasc[:], lah[:])
            wavh = small.tile([128, 8], FP16)
            nc.scalar.dma_start(
                wavh[:],
                bass.AP(tensor=wasc.tensor, offset=0, ap=[[1, 128], [128, 8]]))

            # ---- m7 chain (pse bank, pre-gather) ----
            ps7b = pse.tile([128, 512], F32, name="ps7b", tag="pse")
            for kk in range(KK):
                nc.tensor.matmul(ps7b[:], lhs(7, kk), rhs(kk),
                                 start=(kk == 0), stop=False, perf_mode=DR)

            # ---- inject na/nb into each psum chain, then epilogue ----
            for m in range(8):
                psm = ps7[:, m, :] if m < 7 else ps7b[:]
                nc.tensor.matmul(
                    psm, nk2[:, m // 2, 128 * (m % 2):128 * (m % 2) + 128],
                    nb2[:], start=False, stop=True, skip_group_check=True)
                w2m = w2p.tile([128, 512], F32, name="w2m", tag="w2")
                nc.scalar.activation(w2m[:], wbbc[:],
                                     mybir.ActivationFunctionType.Square,
                                     bias=wavh[:, m:m + 1], scale=1.0)
                ot = outp.tile([128, 512], F32, name="ot", tag="ot")
                nc.vector.scalar_tensor_tensor(
                    ot[:], psm, 0.0, w2m[:], op0=ALU.max, op1=ALU.mult)
                nc.sync.dma_start(out_d[128 * m:128 * (m + 1), :], ot[:])

    nc.compile()
    return nc


def _p_major(a, np_dtype):
    """[T, cols] -> [128, T//128, cols]."""
    n = a.shape[0] // 128
    return np.ascontiguousarray(
        a.reshape(n, 128, a.shape[1]).transpose(1, 0, 2).astype(np_dtype))


def _core_geom(d):
    r, cb = d >> 2, d & 3
    rows = 1024 * r + (256 * cb + np.arange(MB)) % 1024
    cols = 512 * cb + (256 * r + np.arange(NBC)) % 512
    return rows, cols


def kernel(x, fc_weight, _trace=False):
    """Full inputs in, full [2048, 2048] scores out."""
    x = np.asarray(x, dtype=np.float32)
    fc = np.asarray(fc_weight, dtype=np.float32)
    xf = x.reshape(T, C)
    xa2 = np.ascontiguousarray(xf[:, 0::2]) * -2.0   # [T, 2048]
    xb = np.ascontiguousarray(xf[:, 1::2])

    if "v4" not in _cache:
        _cache["v4"] = _build()
    ncv = _cache["v4"]

    in_maps = []
    geoms = []
    for d in range(D):
        rows, cols = _core_geom(d)
        geoms.append((rows, cols))
        xa_blk = xa2[:, rows]
        xb_blk = xb[:, cols]
        fcd = fc[:, 512 * d:512 * (d + 1)]
        fcs = np.concatenate([fcd[:, 0::2], fcd[:, 1::2]], axis=1)  # [O, 512]
        fr = fcs.reshape(12, 8, 128, 512).astype(np.float16)
        vsel = [oc for oc in range(12) if oc % 3 != 2]
        gsel = [oc for oc in range(12) if oc % 3 == 2]
        fcv = np.ascontiguousarray(fr[vsel].transpose(2, 0, 3, 1))  # [128,8,512,8]
        fcg = np.ascontiguousarray(fr[gsel].transpose(2, 0, 1, 3))  # [128,4,8,512]
        in_maps.append({
            "xan": _p_major(xa_blk[:, :256], NP_FP8),
            "xar": _p_major(xa_blk[:, 256:], NP_FP8),
            "xb": _p_major(xb_blk, NP_FP8),
            "fcv": fcv,
            "fcg": fcg,
        })

    res = run_bass_kernel_spmd(ncv, in_maps, core_ids=list(range(D)), trace=_trace)
    out = np.empty((CA, CA), dtype=np.float32)
    for d in range(D):
        rows, cols = geoms[d]
        out[np.ix_(rows, cols)] = res.results[d]["scores"]
    if _trace:
        kernel.last_times = (res.exec_time_ns,)
    return out
